# revision 1
# baseline (speedup 1.0000x reference)
"""GAT attention kernel for 8 trn2 NeuronCores (Bass/Tile).

Math (restructured from the reference to avoid materializing h_j):
    wa1 = W @ a1, wa2 = W @ a2                      (device, once)
    s[n,k]  = x0[n]·wa1 + x[n,k]·wa2                (since h@a1 = x0@(W a1))
    e       = leaky_relu(s, 0.2)
    p       = exp(e) * adj                          (no max-sub: scores are small)
    att     = (p + EPS) / (sum_k p + 16*EPS)        (== uniform 1/16 when row fully masked,
                                                     matching reference softmax of all -9e15)
    xbar[n] = sum_k att[n,k] * x[n,k,:]
    out     = elu((xbar + x0) @ W)                  (since h_prime + h = (xbar + x0)@W)
    elu(z)  = relu(z) + min(exp(z)-1, 0)

Sharding: node dim N padded 50000 -> 50176 = 8 cores * 49 tiles * 128 rows.
Per 128-row tile the 2048 (n,k) pairs form 16 blocks of [128 nk-rows, 128 feat]
held as x_tile[:, b*128:(b+1)*128] (host pre-permutes x accordingly so the DMA
is a single contiguous 1MB transfer per tile).

Per tile:
  DVE : 16x tensor_tensor_reduce (scores dot products), si TTR, reciprocal,
        att = (p+eps)*RZ_rep (STT), attseg = SEGBIG*att (one broadcast TT)
  PE  : si scatter (C-matmul), Z = group-sum (SEG), RZ broadcast (E8),
        x0^T via identity matmul + 16 xbar matmuls (accumulate xbarT in PSUM),
        final (xbar+x0)^T.T @ W
  ACT : Lrelu, Exp, Z+eps copy, PSUM->SBUF copies, exp/relu of final
  GPS : adj int->float cast, p = exp*adj, elu tail
"""

import numpy as np

N, K, F = 50000, 16, 128
ALPHA = 0.2
NCORES = 8
TILE = 128
NTILES = 49
RPC = TILE * NTILES          # rows per core = 6272
BPT = K                      # nk-blocks per tile = 16
XCOLS = BPT * F + F + K      # x blocks + x0 + adj(f32) = 2192
EPS = 1e-12

_NC_CACHE = {}


def _consts_np():
    p = np.arange(128)
    j8 = np.arange(8)
    b16 = np.arange(16)
    ident = np.eye(128, dtype=np.float32)
    # C[n, q] = 1 iff n%8 == q//16   (si scatter: out[q,b] = si[8b + q//16])
    Cm = (p[:, None] % 8 == p[None, :] // 16).astype(np.float32)
    # SEGBIG[q, 8b+j] = 1 iff j == q//16  (pattern repeats over b)
    segbig = (p[:, None] // 16 == (p[None, :] % 8)).astype(np.float32)
    # E8[j, q] = 1 iff q//16 == j (rows 8..127 zero; used as lhsT [8,128])
    e8 = ((p[:, None] < 8) & (p[None, :] // 16 == p[:, None])).astype(np.float32)
    # SEG[q, j] = 1 iff q//16 == j   [128, 8]
    seg = (p[:, None] // 16 == j8[None, :]).astype(np.float32)
    # SEG8[n, b] = 1 iff n//8 == b   [128, 16]
    seg8 = (p[:, None] // 8 == b16[None, :]).astype(np.float32)
    ones = np.ones((128, 128), dtype=np.float32)
    return np.concatenate([ident, Cm, segbig, e8, seg, seg8, ones], axis=1)  # [128, 664]


def _consts_full_np(W, a):
    # consts + W + a1 + a2 packed into one tensor -> one setup DMA -> PE
    # matmuls see a single DMA semaphore lane (walrus allows only one sync
    # wait on an fp32 Matmult).
    return np.ascontiguousarray(
        np.concatenate(
            [_consts_np(), W.astype(np.float32),
             a[:F].astype(np.float32), a[F:].astype(np.float32)], axis=1)
    )  # [128, 794]


def _build_nc(ntiles=NTILES, finalize=True, reps=1):
    import contextlib

    import concourse.mybir as mybir
    import concourse.tile as tile
    from concourse import bacc

    fp = mybir.dt.float32
    i32 = mybir.dt.int32
    AF = mybir.ActivationFunctionType
    OP = mybir.AluOpType

    nc = bacc.Bacc("TRN2")
    # x tile data with x0 and f32-cast adj packed in trailing columns
    xd = nc.dram_tensor("xd", [ntiles, 128, XCOLS], fp, kind="ExternalInput")
    cst = nc.dram_tensor("cst", [128, 794], fp, kind="ExternalInput")
    yd = nc.dram_tensor("yd", [ntiles, 128, F], fp, kind="ExternalOutput")

    with tile.TileContext(nc) as tc:
        with (
            tc.tile_pool(name="const", bufs=1) as constp,
            tc.tile_pool(name="xin", bufs=7) as xin,
            tc.tile_pool(name="small", bufs=4) as small,
            tc.tile_pool(name="big", bufs=3) as big,
            tc.tile_pool(name="yout", bufs=3) as yout,
            # one PSUM pool; per-tag bufs: si 1 + Z 2 + RZrep 2 + mm 3 = 8 banks
            tc.tile_pool(name="ps", bufs=1, space="PSUM") as ps,
        ):
            ps_sm = ps
            ps_mm = ps
            # ---------------- setup (single DMA -> single wait chains) ----
            consts = constp.tile([128, 794], fp)
            nc.sync.dma_start(out=consts, in_=cst[:, :])
            IDENT = consts[:, 0:128]
            Cm = consts[:, 128:256]
            SEGBIG = consts[:, 256:384]
            E8 = consts[:, 384:512]
            SEG = consts[:, 512:520]
            SEG8 = consts[:, 520:536]
            ONES = consts[:, 536:664]
            W_sb = consts[:, 664:792]
            a1_sb = consts[:, 792:793]
            a2_sb = consts[:, 793:794]

            # W^T via identity matmul
            WT_ps = ps_mm.tile([128, 128], fp, tag="mm", bufs=3)
            nc.tensor.matmul(WT_ps, lhsT=W_sb, rhs=IDENT, start=True, stop=True)
            WT_sb = constp.tile([128, 128], fp)
            nc.scalar.activation(out=WT_sb, in_=WT_ps, func=AF.Copy)

            # wa1 = W@a1, wa2 = W@a2 as columns
            wa_ps = ps_sm.tile([128, 2], fp, tag="si", bufs=1)
            nc.tensor.matmul(wa_ps[:, 0:1], lhsT=WT_sb, rhs=a1_sb, start=True, stop=True)
            nc.tensor.matmul(wa_ps[:, 1:2], lhsT=WT_sb, rhs=a2_sb, start=True, stop=True)
            wa_cols = constp.tile([128, 2], fp)
            nc.scalar.activation(out=wa_cols, in_=wa_ps, func=AF.Copy)

            # rows [1,128] = wa^T, then broadcast each row to 128 partitions
            wa_rep = {}
            for i in (0, 1):
                row_ps = ps_sm.tile([1, 128], fp, tag="Z", bufs=2)
                nc.tensor.matmul(row_ps, lhsT=wa_cols[:, i:i + 1], rhs=IDENT,
                                 start=True, stop=True)
                row_sb = constp.tile([1, 128], fp, tag=f"warow{i}")
                nc.scalar.activation(out=row_sb, in_=row_ps, func=AF.Copy)
                rep_ps = ps_mm.tile([128, 128], fp, tag="mm", bufs=3)
                nc.tensor.matmul(rep_ps, lhsT=ONES[0:1, :], rhs=row_sb[0:1, :],
                                 start=True, stop=True)
                rep_sb = constp.tile([128, 128], fp, tag=f"warep{i}")
                nc.scalar.activation(out=rep_sb, in_=rep_ps, func=AF.Copy)
                wa_rep[i] = rep_sb
            wa1_rep, wa2_rep = wa_rep[0], wa_rep[1]

            # ---------------- software-pipelined tile loop ----------------
            # Five phases, each tile's cross-engine dependencies are >= 1
            # round old so no engine stalls on another's round-trip latency:
            #   load(t) | scores(t-2) | mask(t-3) | recip(t-4) | out(t-5)
            st = {}

            def phase_load(t):
                xall = xin.tile([128, XCOLS], fp, tag="x")
                nc.sync.dma_start(out=xall, in_=xd[t])
                st[t] = {"xall": xall}

            def phase_score(t):
                d = st[t]
                xall = d["xall"]
                x0_tile = xall[:, BPT * F:BPT * F + F]
                # si chain starts first: DVE -> Pool -> PE -> ACT completes
                # while DVE streams the 16 score blocks.
                scr = big.tile([128, 128], fp, tag="scr")
                si_nat = small.tile([128, 1], fp, tag="si_nat")
                nc.vector.scalar_tensor_tensor(
                    out=scr, in0=x0_tile, scalar=1.0, in1=wa1_rep,
                    op0=OP.mult, op1=OP.mult, accum_out=si_nat,
                )
                Dt = small.tile([128, K], fp, tag="D")
                nc.gpsimd.tensor_scalar_mul(out=Dt, in0=SEG8, scalar1=si_nat)
                si_ps = ps_sm.tile([128, K], fp, tag="si", bufs=1)
                nc.tensor.matmul(si_ps, lhsT=Cm, rhs=Dt, start=True, stop=True)
                si_s = small.tile([128, K], fp, tag="si_s")
                nc.scalar.activation(out=si_s, in_=si_ps, func=AF.Copy)

                s = small.tile([128, K], fp, tag="s")
                for b in range(BPT):
                    nc.vector.scalar_tensor_tensor(
                        out=scr, in0=xall[:, b * F:(b + 1) * F], scalar=1.0,
                        in1=wa2_rep, op0=OP.mult, op1=OP.mult,
                        accum_out=s[:, b:b + 1],
                    )
                s2 = small.tile([128, K], fp, tag="s2")
                nc.gpsimd.tensor_add(out=s2, in0=s, in1=si_s)
                d["s2"] = s2

            def phase_mask(t):
                d = st[t]
                adj_f = d["xall"][:, BPT * F + F:XCOLS]
                ls = small.tile([128, K], fp, tag="ls")
                nc.vector.scalar_tensor_tensor(
                    out=ls, in0=d["s2"], scalar=ALPHA, in1=d["s2"],
                    op0=OP.mult, op1=OP.max,
                )
                exp_s = small.tile([128, K], fp, tag="exp_s")
                nc.scalar.activation(out=exp_s, in_=ls, func=AF.Exp)
                p_s = small.tile([128, K], fp, tag="p_s")
                nc.gpsimd.tensor_mul(out=p_s, in0=exp_s, in1=adj_f)
                Z_ps = ps_sm.tile([8, K], fp, tag="Z", bufs=2)
                nc.tensor.matmul(Z_ps, lhsT=SEG, rhs=p_s, start=True, stop=True)
                tz = small.tile([8, K], fp, tag="tz")
                nc.scalar.activation(out=tz, in_=Z_ps, func=AF.Copy, bias=16.0 * EPS)
                d["p_s"] = p_s
                d["tz"] = tz

            def phase_recip(t):
                d = st[t]
                RZ = small.tile([8, K], fp, tag="RZ")
                nc.vector.reciprocal(RZ, d["tz"])
                RZrep_ps = ps_sm.tile([128, K], fp, tag="RZrep", bufs=2)
                nc.tensor.matmul(RZrep_ps, lhsT=E8[0:8, :], rhs=RZ,
                                 start=True, stop=True)
                d["RZrep"] = RZrep_ps

            def phase_out(t):
                d = st[t]
                xall = d["xall"]
                x0_tile = xall[:, BPT * F:BPT * F + F]
                att = small.tile([128, K], fp, tag="att")
                nc.vector.scalar_tensor_tensor(
                    out=att, in0=d["p_s"], scalar=EPS, in1=d["RZrep"],
                    op0=OP.add, op1=OP.mult,
                )
                attseg = big.tile([128, 128], fp, tag="attseg")
                att_bc = att.rearrange("p (b o) -> p b o", o=1).to_broadcast([128, K, 8])
                nc.vector.tensor_mul(
                    out=attseg.rearrange("p (b j) -> p b j", j=8),
                    in0=SEGBIG.rearrange("p (b j) -> p b j", j=8),
                    in1=att_bc,
                )
                xbarT_ps = ps_mm.tile([128, 128], fp, tag="mm", bufs=3)
                nc.tensor.matmul(xbarT_ps, lhsT=x0_tile, rhs=IDENT,
                                 start=True, stop=False, skip_group_check=True)
                for b in range(BPT):
                    nc.tensor.matmul(
                        xbarT_ps[:, 8 * b:8 * b + 8],
                        lhsT=xall[:, b * F:(b + 1) * F],
                        rhs=attseg[:, 8 * b:8 * b + 8],
                        start=False, stop=(b == BPT - 1), skip_group_check=True,
                    )
                ST_sb = big.tile([128, 128], fp, tag="ST")
                nc.scalar.activation(out=ST_sb, in_=xbarT_ps, func=AF.Copy)
                zfin_ps = ps_mm.tile([128, 128], fp, tag="mm", bufs=3)
                nc.tensor.matmul(zfin_ps, lhsT=ST_sb, rhs=W_sb, start=True, stop=True)
                e_sb = big.tile([128, 128], fp, tag="e")
                nc.scalar.activation(out=e_sb, in_=zfin_ps, func=AF.Exp)
                r_sb = big.tile([128, 128], fp, tag="r")
                nc.scalar.activation(out=r_sb, in_=zfin_ps, func=AF.Relu)
                u_sb = big.tile([128, 128], fp, tag="u")
                nc.gpsimd.tensor_scalar(
                    out=u_sb, in0=e_sb, scalar1=1.0, scalar2=0.0,
                    op0=OP.subtract, op1=OP.min,
                )
                y_sb = yout.tile([128, 128], fp, tag="y")
                nc.gpsimd.tensor_add(out=y_sb, in0=r_sb, in1=u_sb)
                nc.sync.dma_start(out=yd[t], in_=y_sb)
                del st[t]

            rep_ctx = tc.For_i(0, reps, 1) if reps > 1 else contextlib.nullcontext()
            with rep_ctx:
                for r in range(ntiles + 5):
                    if r < ntiles:
                        phase_load(r)
                    if 0 <= r - 2 < ntiles:
                        phase_score(r - 2)
                    if 0 <= r - 3 < ntiles:
                        phase_mask(r - 3)
                    if 0 <= r - 4 < ntiles:
                        phase_recip(r - 4)
                    if 0 <= r - 5 < ntiles:
                        phase_out(r - 5)

    if finalize:
        nc.finalize()
    return nc


def _get_nc(ntiles=NTILES):
    if ntiles not in _NC_CACHE:
        _NC_CACHE[ntiles] = _build_nc(ntiles)
    return _NC_CACHE[ntiles]


def _shard_inputs(orignal_x, x, adj, W, a, ncores=NCORES, ntiles=NTILES):
    f32 = np.float32
    rpc = TILE * ntiles
    n_used = rpc * ncores
    x = np.asarray(x, f32)
    x0 = np.asarray(orignal_x, f32)
    adj = np.asarray(adj, np.int32)
    consts = _consts_full_np(np.asarray(W, f32), np.asarray(a, f32))
    n = x.shape[0]

    in_maps = []
    for c in range(ncores):
        lo = c * rpc
        hi = min((c + 1) * rpc, n)
        rows = hi - lo
        xc = x[lo:hi]
        x0c = x0[lo:hi]
        adjc = adj[lo:hi]
        if rows < rpc:
            pad = rpc - rows
            xc = np.concatenate([xc, np.zeros((pad, K, F), f32)])
            x0c = np.concatenate([x0c, np.zeros((pad, F), f32)])
            adjc = np.concatenate([adjc, np.zeros((pad, K), np.int32)])
        # per-tile layout [t, p, b*F+f] with x0 and f32 adj packed as
        # trailing columns (adj in s-layout: adj_s[q, b] = adj_flat[128b+q])
        xdev = np.empty((ntiles, 128, XCOLS), f32)
        xdev[:, :, :BPT * F] = xc.reshape(ntiles, BPT, 128, F).transpose(
            0, 2, 1, 3).reshape(ntiles, 128, BPT * F)
        xdev[:, :, BPT * F:BPT * F + F] = x0c.reshape(ntiles, 128, F)
        xdev[:, :, BPT * F + F:] = adjc.reshape(ntiles, BPT, 128).transpose(
            0, 2, 1).astype(f32)
        in_maps.append({
            "xd": xdev,
            "cst": consts,
        })
    assert n <= n_used
    return in_maps


_LAST_RESULTS = None


def kernel(orignal_x, x, adj, W, a):
    import os
    os.environ.setdefault("JAX_PLATFORMS", "")
    from concourse.bass_utils import run_bass_kernel_spmd

    global _LAST_RESULTS
    nc = _get_nc()
    in_maps = _shard_inputs(orignal_x, x, adj, W, a)
    res = run_bass_kernel_spmd(nc, in_maps, list(range(NCORES)))
    _LAST_RESULTS = res
    y = np.concatenate([r["yd"].reshape(RPC, F) for r in res.results], axis=0)
    return np.ascontiguousarray(y[:N])



# revision 3
# speedup vs baseline: 3.1832x; 3.1832x over previous
"""GAT attention kernel for 8 trn2 NeuronCores (Bass/Tile), bf16 q-layout.

Math (restructured from the reference):
    wa1 = W @ a1, wa2 = W @ a2                      (host, weight folding)
    x'  = x * wa2[f]   (host)                        so  sj[n,k] = sum_f x'[n,k,f]
    x0w1 = x0 * wa1[f] (host)                        so  si[n]   = sum_f x0w1[n,f]
    x0' = x0 * wa2[f]  (host),  W' = W / wa2[:,None] (host)
    s       = si + sj
    p       = exp(prelu(s, 0.2)) * adj
    Z'      = sum_k p + 16*EPS                       (per node)
    U       = sum_k (p_k+EPS) * x'_k  +  Z' * x0'    (unnormalized, PSUM)
    out     = elu((U @ W') / Z')                     (/Z' via ACT per-partition scale)
    elu(z)  = relu(z) - relu(1 - exp(z))

Sharding: node dim N padded 50000 -> 50176 = 8 cores * 49 tiles * 128 rows.
Per 128-row tile the 2048 (n,k) pairs form 16 blocks of [128 q, 128 f] in
bf16 (q = 16*(n%8) + k, block b = n_tile//8), followed by x0w1 (natural
[n,f]), x0' and adj in s-layout, in ONE contiguous DMA per tile.

Engines per tile:
  DVE : 2 halving TTs + segmented tensor_reduce (all 17 score dots in 3 ops),
        x0z = x0'*Z' (tensor_scalar), attsegU = (p+EPS)*SEGBIG (STT),
        rz = 1/Z' (approx recip), y = r - v (TT)
  PE  : si scatter (Cm), ZT rowsum, 17 accumulating xbar matmuls (bf16,
        x-as-lhsT -> U^T directly), final U^T.T @ W'
  ACT : prelu, exp (scores), tz bias, U^T copy to SBUF, exp/relu/relu of elu
        (all in one act table: exp_and_others)
  GPS : si broadcast (Dt), s2 add, p mask
  DMA : 594KB in + 32KB out per tile + [16,8]->[128,1] Z scatter
"""

import numpy as np
import ml_dtypes

N, K, F = 50000, 16, 128
ALPHA = 0.2
NCORES = 8
TILE = 128
NTILES = 49
RPC = TILE * NTILES          # rows per core = 6272
BPT = K                      # nk-blocks per tile = 16
# bf16 columns: 16 x' blocks | x0w1 | x0' | adj_s
XCOLS = BPT * F + F + F + K  # 2320
EPS = 1e-12

BF16 = ml_dtypes.bfloat16

_NC_CACHE = {}


def _consts_np():
    p = np.arange(128)
    j8 = np.arange(8)
    b16 = np.arange(16)
    ident = np.eye(128, dtype=np.float32)
    # Cm[p, q] = 1 iff p%8 == q//16   (si scatter: si_s[q,b] = si[8b + q//16])
    Cm = (p[:, None] % 8 == p[None, :] // 16).astype(np.float32)
    # SEGBIG[q, 8b+j] = 1 iff j == q//16
    segbig = (p[:, None] // 16 == (p[None, :] % 8)).astype(np.float32)
    # SEG[q, j] = 1 iff q//16 == j   [128, 8]  (ZT rowsum rhs)
    seg = (p[:, None] // 16 == j8[None, :]).astype(np.float32)
    # SEG8[n, b] = 1 iff n//8 == b   [128, 16] (si broadcast on GPS)
    seg8 = (p[:, None] // 8 == b16[None, :]).astype(np.float32)
    return ident, Cm, segbig, seg, seg8


def _consts_full_np(W, a):
    W = np.asarray(W, np.float64)
    a = np.asarray(a, np.float64)
    wa1 = W @ a[:F, 0]
    wa2 = W @ a[F:, 0]
    Wp = W / wa2[:, None]
    ident, Cm, segbig, seg, seg8 = _consts_np()
    cst = np.concatenate(
        [segbig, Cm, ident, Wp.astype(np.float32), seg8, seg], axis=1)
    return np.ascontiguousarray(cst).astype(BF16), wa1, wa2  # [128, 536]


def _build_nc(ntiles=NTILES, finalize=True):
    import concourse.mybir as mybir
    import concourse.tile as tile
    from concourse import bacc

    fp = mybir.dt.float32
    bf = mybir.dt.bfloat16
    AF = mybir.ActivationFunctionType
    OP = mybir.AluOpType
    AX = mybir.AxisListType

    nc = bacc.Bacc("TRN2")
    xd = nc.dram_tensor("xd", [ntiles, 128, XCOLS], bf, kind="ExternalInput")
    cst = nc.dram_tensor("cst", [128, 536], bf, kind="ExternalInput")
    yd = nc.dram_tensor("yd", [ntiles, 128, F], bf, kind="ExternalOutput")

    with tile.TileContext(nc) as tc:
        with (
            tc.tile_pool(name="const", bufs=1) as constp,
            tc.tile_pool(name="xin", bufs=7) as xin,
            tc.tile_pool(name="sm", bufs=4) as sm,
            tc.tile_pool(name="med", bufs=3) as med,
            tc.tile_pool(name="big", bufs=3) as big,
            tc.tile_pool(name="yout", bufs=3) as yout,
            tc.tile_pool(name="ps", bufs=1, space="PSUM") as ps,
        ):
            consts = constp.tile([128, 536], bf)
            nc.sync.dma_start(out=consts, in_=cst[:, :])
            SEGBIG = consts[:, 0:128]
            Cm = consts[:, 128:256]
            IDENT = consts[:, 256:384]
            Wp = consts[:, 384:512]
            SEG8 = consts[:, 512:528]
            SEG = consts[:, 528:536]

            st = {}

            def phase_load(t):
                xall = xin.tile([128, XCOLS], bf, tag="x")
                nc.sync.dma_start(out=xall, in_=xd[t])
                st[t] = {"xall": xall}

            def phase_score(t):
                d = st[t]
                xall = d["xall"]
                # 17-segment score sums: 16 x' blocks + x0w1 (si), two
                # halving adds then a segmented reduce.
                h1 = med.tile([128, 17 * 64], bf, tag="h1")
                s3 = xall[:, 0:17 * F].rearrange("p (s f) -> p s f", f=F)
                nc.vector.tensor_tensor(
                    out=h1.rearrange("p (s f) -> p s f", f=64),
                    in0=s3[:, :, 0:64], in1=s3[:, :, 64:128], op=OP.add)
                h2 = med.tile([128, 17 * 32], bf, tag="h2")
                h1v = h1.rearrange("p (s f) -> p s f", f=64)
                nc.vector.tensor_tensor(
                    out=h2.rearrange("p (s f) -> p s f", f=32),
                    in0=h1v[:, :, 0:32], in1=h1v[:, :, 32:64], op=OP.add)
                s17 = sm.tile([128, 17], fp, tag="s17")
                nc.vector.tensor_reduce(
                    out=s17, in_=h2.rearrange("p (s f) -> p s f", f=32),
                    axis=AX.X, op=OP.add)
                # scatter si (natural [n,1]) into s-layout via SEG8 + Cm
                Dt = sm.tile([128, K], bf, tag="Dt")
                nc.gpsimd.tensor_scalar_mul(out=Dt, in0=SEG8,
                                            scalar1=s17[:, 16:17])
                si_ps = ps.tile([128, K], fp, tag="si", bufs=2)
                nc.tensor.matmul(si_ps, lhsT=Cm, rhs=Dt, start=True, stop=True)
                s2 = sm.tile([128, K], fp, tag="s2")
                nc.vector.scalar_tensor_tensor(
                    out=s2, in0=s17[:, 0:16], scalar=0.0, in1=si_ps,
                    op0=OP.add, op1=OP.add)
                ls = sm.tile([128, K], fp, tag="ls")
                nc.scalar.activation(out=ls, in_=s2, func=AF.Prelu, alpha=ALPHA)
                exp_s = sm.tile([128, K], bf, tag="exp_s")
                nc.scalar.activation(out=exp_s, in_=ls, func=AF.Exp)
                p_s = sm.tile([128, K], bf, tag="p_s")
                nc.gpsimd.tensor_mul(out=p_s, in0=exp_s,
                                     in1=xall[:, BPT * F + 2 * F:XCOLS])
                d["p_s"] = p_s

            def phase_z(t):
                d = st[t]
                ZT_ps = ps.tile([16, 8], fp, tag="zt", bufs=2)
                nc.tensor.matmul(ZT_ps, lhsT=d["p_s"], rhs=SEG,
                                 start=True, stop=True)
                tz = sm.tile([16, 8], fp, tag="tz")
                nc.scalar.activation(out=tz, in_=ZT_ps, func=AF.Copy,
                                     bias=16.0 * EPS)
                zn = sm.tile([128, 1], fp, tag="zn")
                nc.sync.dma_start(out=zn, in_=tz)
                rz = sm.tile([128, 1], fp, tag="rz")
                nc.vector.reciprocal_approx_fast(rz, zn)
                d["zn"] = zn
                d["rz"] = rz

            def phase_xbar(t):
                d = st[t]
                xall = d["xall"]
                x0z = big.tile([128, F], bf, tag="x0z")
                nc.vector.tensor_scalar(
                    out=x0z, in0=xall[:, BPT * F + F:BPT * F + 2 * F],
                    scalar1=d["zn"], scalar2=None, op0=OP.mult)
                attsegU = big.tile([128, 128], bf, tag="attsegU")
                p_bc = d["p_s"].rearrange("p (b o) -> p b o", o=1)
                nc.vector.scalar_tensor_tensor(
                    out=attsegU.rearrange("p (b j) -> p b j", j=8),
                    in0=p_bc.to_broadcast([128, K, 8]), scalar=EPS,
                    in1=SEGBIG.rearrange("p (b j) -> p b j", j=8),
                    op0=OP.add, op1=OP.mult)
                xbarT_ps = ps.tile([128, 128], fp, tag="mm", bufs=3)
                nc.tensor.matmul(xbarT_ps, lhsT=x0z, rhs=IDENT,
                                 start=True, stop=False, skip_group_check=True)
                for b in range(BPT):
                    nc.tensor.matmul(
                        xbarT_ps[:, 8 * b:8 * b + 8],
                        lhsT=xall[:, b * F:(b + 1) * F],
                        rhs=attsegU[:, 8 * b:8 * b + 8],
                        start=False, stop=(b == BPT - 1), skip_group_check=True)
                ST = big.tile([128, 128], bf, tag="ST")
                nc.scalar.activation(out=ST, in_=xbarT_ps, func=AF.Copy)
                d["ST"] = ST

            def phase_out(t):
                d = st[t]
                rz = d["rz"]
                Y_ps = ps.tile([128, 128], fp, tag="mm", bufs=3)
                nc.tensor.matmul(Y_ps, lhsT=d["ST"], rhs=Wp,
                                 start=True, stop=True)
                e = yout.tile([128, 128], bf, tag="e")
                nc.scalar.activation(out=e, in_=Y_ps, func=AF.Exp, scale=rz)
                r = yout.tile([128, 128], bf, tag="r")
                nc.scalar.activation(out=r, in_=Y_ps, func=AF.Relu, scale=rz)
                v = yout.tile([128, 128], bf, tag="v")
                nc.scalar.activation(out=v, in_=e, func=AF.Relu,
                                     scale=-1.0, bias=1.0)
                y = yout.tile([128, 128], bf, tag="y")
                nc.vector.tensor_tensor(out=y, in0=r, in1=v, op=OP.subtract)
                nc.sync.dma_start(out=yd[t], in_=y)
                del st[t]

            for r in range(ntiles + 5):
                if r < ntiles:
                    phase_load(r)
                if 0 <= r - 2 < ntiles:
                    phase_score(r - 2)
                if 0 <= r - 3 < ntiles:
                    phase_z(r - 3)
                if 0 <= r - 4 < ntiles:
                    phase_xbar(r - 4)
                if 0 <= r - 5 < ntiles:
                    phase_out(r - 5)

    if finalize:
        nc.finalize()
    return nc


def _get_nc(ntiles=NTILES):
    if ntiles not in _NC_CACHE:
        _NC_CACHE[ntiles] = _build_nc(ntiles)
    return _NC_CACHE[ntiles]


def _shard_inputs(orignal_x, x, adj, W, a, ncores=NCORES, ntiles=NTILES):
    f32 = np.float32
    rpc = TILE * ntiles
    n_used = rpc * ncores
    x = np.asarray(x, f32)
    x0 = np.asarray(orignal_x, f32)
    adj = np.asarray(adj, np.int32)
    cst, wa1, wa2 = _consts_full_np(W, a)
    wa1_f = wa1.astype(f32)
    wa2_f = wa2.astype(f32)
    n = x.shape[0]
    assert n <= n_used

    in_maps = []
    for c in range(ncores):
        lo = c * rpc
        hi = min((c + 1) * rpc, n)
        rows = hi - lo
        xc = x[lo:hi]
        x0c = x0[lo:hi]
        adjc = adj[lo:hi]
        if rows < rpc:
            pad = rpc - rows
            xc = np.concatenate([xc, np.zeros((pad, K, F), f32)])
            x0c = np.concatenate([x0c, np.zeros((pad, F), f32)])
            adjc = np.concatenate([adjc, np.zeros((pad, K), np.int32)])
        xdev = np.empty((ntiles, 128, XCOLS), BF16)
        # x' blocks in q-layout: [t, b, j, k, f] -> [t, (j,k), (b,f)]
        xp = (xc * wa2_f[None, None, :]).astype(BF16)
        xdev[:, :, :BPT * F] = xp.reshape(ntiles, 16, 8, K, F).transpose(
            0, 2, 3, 1, 4).reshape(ntiles, 128, BPT * F)
        xdev[:, :, BPT * F:BPT * F + F] = (
            x0c * wa1_f[None, :]).astype(BF16).reshape(ntiles, 128, F)
        xdev[:, :, BPT * F + F:BPT * F + 2 * F] = (
            x0c * wa2_f[None, :]).astype(BF16).reshape(ntiles, 128, F)
        # adj in s-layout: [t, b, j, k] -> [t, (j,k), b]
        xdev[:, :, BPT * F + 2 * F:] = adjc.astype(BF16).reshape(
            ntiles, 16, 8, K).transpose(0, 2, 3, 1).reshape(ntiles, 128, K)
        in_maps.append({"xd": xdev, "cst": cst})
    return in_maps


_LAST_RESULTS = None


def kernel(orignal_x, x, adj, W, a):
    import os
    os.environ.setdefault("JAX_PLATFORMS", "")
    from concourse.bass_utils import run_bass_kernel_spmd

    global _LAST_RESULTS
    nc = _get_nc()
    in_maps = _shard_inputs(orignal_x, x, adj, W, a)
    res = run_bass_kernel_spmd(nc, in_maps, list(range(NCORES)))
    _LAST_RESULTS = res
    y = np.concatenate(
        [np.asarray(r["yd"]).astype(np.float32).reshape(RPC, F)
         for r in res.results], axis=0)
    return np.ascontiguousarray(y[:N])
